# revision 15
# baseline (speedup 1.0000x reference)
"""Trainium2 Bass kernel for nn_InterferenceDecoder.

out[s, v] = |sum_e conj(psi)[s,e] * patterns[v,e]|^2 + (psi_real @ W.T)[s, v] + b[v]

Strategy (8 NeuronCores, tensor-parallel on vocab):
  - vocab 50257 padded to 51200 = 8 * 6400; core i owns vocab slab [i*6400, (i+1)*6400)
  - psi replicated; patterns/W/b sharded on vocab; operands pre-transposed
    on host so the contraction dim E=128 is the SBUF partition dim
  - per [128v x 512s] tile, 5 fp16 matmuls (measured at the PE's
    throughput floor; fp8-DoubleRow loses its 2x to a serialized ~125ns
    LDWEIGHTS since 256-col dual-fp8 weights occupy both PE weight
    buffers, and matmul outputs are capped at 512 elements so the load
    cannot amortize over a wider moving stream):
      psum_ri[0:512]  = patR.psiR + patI.psiI        (Re)
      psum_ri[512:]   = patI.psiR + patR.(-psiI)     (Im)
      psum_l[si-half] = W.psiR                       (linear)
  - elementwise, per s-pair (s12q is a per-v-tile [128, 4, 1024] f16
    scratch holding [Re^2 | Im^2] per s-tile):
      s12q[s]  = Square(psum_ri)          (ACT -> f16; every Nth tile does
                                           DVE copy + f16 self-multiply
                                           instead, to balance engines)
      stage    = (psum_l + b_v) + s12q[Re-view]   (DVE stt, 1024 wide)
      stage   += s12q[Im-view]                    (DVE tt, f16 4x mode)
  - output written f16 ([6400, 2048] per core) halving write traffic;
    host upcasts to f32, transposes, concatenates, slices off padding.
"""

import sys

for _p in ("/opt/trn_rl_repo", "/opt/pypackages"):
    if _p not in sys.path:
        sys.path.append(_p)

import numpy as np

import concourse.bass as bass
import concourse.mybir as mybir
from concourse import bacc
from concourse.tile import TileContext
from concourse.bass_utils import run_bass_kernel_spmd


def _install_ntff_hook_shim():
    """Provide antenv.axon_hooks if the image lacks it, so trace=True can
    capture NTFF profiles through the axon PJRT .so."""
    try:
        from antenv import axon_hooks  # noqa: F401
        return
    except ImportError:
        pass
    import contextlib
    import ctypes
    import types

    import antenv

    so_path = "/opt/axon/libaxon_pjrt.so"
    mod = types.ModuleType("antenv.axon_hooks")
    _state = {"hook": None}

    def set_axon_ntff_profile_hook(hook):
        _state["hook"] = hook

    def get_axon_ntff_profile_hook():
        return _state["hook"]

    mod.set_axon_ntff_profile_hook = set_axon_ntff_profile_hook
    mod.get_axon_ntff_profile_hook = get_axon_ntff_profile_hook
    sys.modules["antenv.axon_hooks"] = mod
    antenv.axon_hooks = mod

    try:
        lib = ctypes.CDLL(so_path)
    except OSError:
        return
    if not hasattr(lib, "axon_start_nrt_profile"):
        return
    lib.axon_start_nrt_profile.argtypes = [
        ctypes.POINTER(ctypes.c_int64), ctypes.c_size_t]
    lib.axon_start_nrt_profile.restype = ctypes.c_int64
    lib.axon_stop_nrt_profile.argtypes = [ctypes.c_char_p]
    lib.axon_stop_nrt_profile.restype = ctypes.c_int64

    @contextlib.contextmanager
    def _hook(output_dir, device_ids):
        import jax
        jax.devices()
        if device_ids:
            ids = (ctypes.c_int64 * len(device_ids))(*device_ids)
            rc = lib.axon_start_nrt_profile(ids, len(device_ids))
        else:
            rc = lib.axon_start_nrt_profile(None, 0)
        if rc != 0:
            raise RuntimeError(f"axon_start_nrt_profile rc={rc}")
        try:
            yield
        finally:
            n = lib.axon_stop_nrt_profile(str(output_dir).encode())
            print(f"ntff profile: {n} file(s) written to {output_dir}",
                  file=sys.stderr)

    set_axon_ntff_profile_hook(_hook)


_install_ntff_hook_shim()

SEQ = 2048
EMBED = 128
VOCAB = 50257
N_CORES = 8
V_PAD = 51200            # 8 * 6400
V_CORE = V_PAD // N_CORES  # 6400
V_TILES = V_CORE // 128    # 50
S_TILE = 512
S_TILES = SEQ // S_TILE    # 4

F32 = mybir.dt.float32
F16 = mybir.dt.float16

# every DVE_SQ_MOD'th (v, s) tile squares its psum on DVE (copy + f16
# self-multiply) instead of ACT; 0 disables (DVE psum-copies release the
# psum banks slower than ACT squares, stalling the matmul pipeline)
DVE_SQ_MOD = 16
DVE_SQ_REM = 3

_compiled = {}


def _build_program():
    nc = bacc.Bacc()

    pat_r = nc.dram_tensor("pat_r", [EMBED, V_CORE], F16, kind="ExternalInput")
    pat_i = nc.dram_tensor("pat_i", [EMBED, V_CORE], F16, kind="ExternalInput")
    w_t = nc.dram_tensor("w_t", [EMBED, V_CORE], F16, kind="ExternalInput")
    psi_r = nc.dram_tensor("psi_r", [EMBED, SEQ], F16, kind="ExternalInput")
    psi_i = nc.dram_tensor("psi_i", [EMBED, SEQ], F16, kind="ExternalInput")
    psi_in = nc.dram_tensor("psi_in", [EMBED, SEQ], F16, kind="ExternalInput")
    b_rs = nc.dram_tensor("b_rs", [128, V_TILES], F32, kind="ExternalInput")
    out_t = nc.dram_tensor("out_t", [V_CORE, SEQ], F16, kind="ExternalOutput")

    add = mybir.AluOpType.add
    mult = mybir.AluOpType.mult

    with TileContext(nc) as tc:
        with tc.tile_pool(name="weights", bufs=1) as wpool, \
             tc.tile_pool(name="stage", bufs=3) as stpool, \
             tc.tile_pool(name="eltw", bufs=2) as epool, \
             tc.tile_pool(name="psum", bufs=2, space="PSUM") as pspool, \
             tc.tile_pool(name="psum_l", bufs=2, space="PSUM") as plpool:
            patr_sb = wpool.tile([EMBED, V_CORE], F16)
            pati_sb = wpool.tile([EMBED, V_CORE], F16)
            wt_sb = wpool.tile([EMBED, V_CORE], F16)
            psir_sb = wpool.tile([EMBED, SEQ], F16)
            psii_sb = wpool.tile([EMBED, SEQ], F16)
            psin_sb = wpool.tile([EMBED, SEQ], F16)
            b_sb = wpool.tile([128, V_TILES], F32)

            # load order: the first s-tile of psi + one v-tile of weights
            # unblock the first tile's matmuls within ~1us, then the rest
            sf = slice(0, S_TILE)
            nc.sync.dma_start(out=psir_sb[:, sf], in_=psi_r[:, sf])
            first = slice(0, 128)
            nc.sync.dma_start(out=patr_sb[:, first], in_=pat_r[:, first])
            nc.sync.dma_start(out=psii_sb[:, sf], in_=psi_i[:, sf])
            nc.sync.dma_start(out=pati_sb[:, first], in_=pat_i[:, first])
            nc.sync.dma_start(out=psin_sb[:, sf], in_=psi_in[:, sf])
            nc.sync.dma_start(out=wt_sb[:, first], in_=w_t[:, first])
            nc.sync.dma_start(out=b_sb[:], in_=b_rs[:])
            nc.sync.dma_start(out=psir_sb[:, S_TILE:], in_=psi_r[:, S_TILE:])
            nc.sync.dma_start(out=psii_sb[:, S_TILE:], in_=psi_i[:, S_TILE:])
            nc.sync.dma_start(out=psin_sb[:, S_TILE:], in_=psi_in[:, S_TILE:])
            first = slice(128, 2 * 128)
            nc.sync.dma_start(out=patr_sb[:, first], in_=pat_r[:, first])
            nc.sync.dma_start(out=pati_sb[:, first], in_=pat_i[:, first])
            nc.sync.dma_start(out=wt_sb[:, first], in_=w_t[:, first])
            CHUNK = 8 * 128
            for c0 in range(2 * 128, V_CORE, CHUNK):
                cs = slice(c0, min(c0 + CHUNK, V_CORE))
                nc.sync.dma_start(out=patr_sb[:, cs], in_=pat_r[:, cs])
                nc.sync.dma_start(out=pati_sb[:, cs], in_=pat_i[:, cs])
                nc.sync.dma_start(out=wt_sb[:, cs], in_=w_t[:, cs])

            for v in range(V_TILES):
                vs = slice(v * 128, (v + 1) * 128)
                stage = stpool.tile([128, S_TILES, S_TILE], F16, tag="stage")
                s12q = epool.tile([128, S_TILES, 2 * S_TILE], F16, tag="sq")
                for sp in range(S_TILES // 2):
                    psum_l = plpool.tile([128, 2, S_TILE], F32, tag="lin")
                    for si in range(2):
                        s = 2 * sp + si
                        ss = slice(s * S_TILE, (s + 1) * S_TILE)
                        psum_ri = pspool.tile([128, 2 * S_TILE], F32,
                                              tag="ri", name=f"ri_{v}_{s}")
                        nc.tensor.matmul(psum_ri[:, 0:S_TILE],
                                         patr_sb[:, vs], psir_sb[:, ss],
                                         start=True, stop=False)
                        nc.tensor.matmul(psum_ri[:, 0:S_TILE],
                                         pati_sb[:, vs], psii_sb[:, ss],
                                         start=False, stop=True)
                        nc.tensor.matmul(psum_ri[:, S_TILE:],
                                         pati_sb[:, vs], psir_sb[:, ss],
                                         start=True, stop=False)
                        nc.tensor.matmul(psum_ri[:, S_TILE:],
                                         patr_sb[:, vs], psin_sb[:, ss],
                                         start=False, stop=True)
                        nc.tensor.matmul(psum_l[:, si, :], wt_sb[:, vs],
                                         psir_sb[:, ss],
                                         start=True, stop=True)
                        if DVE_SQ_MOD and \
                                (v * S_TILES + s) % DVE_SQ_MOD == DVE_SQ_REM:
                            c_ri = epool.tile([128, 2 * S_TILE], F16,
                                              tag="cri")
                            nc.vector.tensor_copy(out=c_ri[:],
                                                  in_=psum_ri[:])
                            nc.vector.tensor_tensor(
                                out=s12q[:, s, :], in0=c_ri[:], in1=c_ri[:],
                                op=mult)
                        else:
                            nc.scalar.square(s12q[:, s, :], psum_ri[:])
                    pair = slice(2 * sp, 2 * sp + 2)
                    nc.vector.scalar_tensor_tensor(
                        out=stage[:, pair, :], in0=psum_l[:],
                        scalar=b_sb[:, v:v + 1],
                        in1=s12q[:, pair, 0:S_TILE],
                        op0=add, op1=add)
                    nc.vector.tensor_tensor(
                        out=stage[:, pair, :], in0=stage[:, pair, :],
                        in1=s12q[:, pair, S_TILE:], op=add)
                    nc.sync.dma_start(
                        out=out_t[vs, sp * 2 * S_TILE:(sp + 1) * 2 * S_TILE],
                        in_=stage[:, pair, :])

    nc.finalize()
    return nc


def _get_program():
    if "nc" not in _compiled:
        _compiled["nc"] = _build_program()
    return _compiled["nc"]


def kernel(psi_real, psi_imag, patterns_real, patterns_imag, W, b):
    psi_real = np.ascontiguousarray(psi_real, dtype=np.float32)
    psi_imag = np.ascontiguousarray(psi_imag, dtype=np.float32)

    psiT_r = np.ascontiguousarray(psi_real.T.astype(np.float16))
    psiT_i = np.ascontiguousarray(psi_imag.T.astype(np.float16))
    psiT_in = np.ascontiguousarray((-psi_imag.T).astype(np.float16))

    def pad_t(m):
        full = np.zeros((EMBED, V_PAD), dtype=np.float16)
        full[:, :VOCAB] = np.asarray(m, dtype=np.float32).T.astype(np.float16)
        return full

    patT_r = pad_t(patterns_real)
    patT_i = pad_t(patterns_imag)
    wT = pad_t(W)
    b_pad = np.zeros((V_PAD,), dtype=np.float32)
    b_pad[:VOCAB] = np.asarray(b, dtype=np.float32)

    in_maps = []
    for c in range(N_CORES):
        vs = slice(c * V_CORE, (c + 1) * V_CORE)
        b_shard = b_pad[vs]
        in_maps.append({
            "pat_r": np.ascontiguousarray(patT_r[:, vs]),
            "pat_i": np.ascontiguousarray(patT_i[:, vs]),
            "w_t": np.ascontiguousarray(wT[:, vs]),
            "psi_r": psiT_r,
            "psi_i": psiT_i,
            "psi_in": psiT_in,
            "b_rs": np.ascontiguousarray(b_shard.reshape(V_TILES, 128).T),
        })

    nc = _get_program()
    res = run_bass_kernel_spmd(nc, in_maps, core_ids=list(range(N_CORES)))
    kernel.last_results = res

    out = np.empty((SEQ, V_PAD), dtype=np.float32)
    for c in range(N_CORES):
        out[:, c * V_CORE:(c + 1) * V_CORE] = \
            res.results[c]["out_t"].T.astype(np.float32)
    return out[:, :VOCAB]


# revision 16
# speedup vs baseline: 1.2223x; 1.2223x over previous
"""Trainium2 Bass kernel for nn_InterferenceDecoder.

out[s, v] = |sum_e conj(psi)[s,e] * patterns[v,e]|^2 + (psi_real @ W.T)[s, v] + b[v]

Strategy (8 NeuronCores, tensor-parallel on vocab):
  - vocab 50257 padded to 51200 = 8 * 6400; core i owns vocab slab [i*6400, (i+1)*6400)
  - psi replicated; patterns/W/b sharded on vocab; operands pre-transposed
    on host so the contraction dim E=128 is the SBUF partition dim
  - per [128v x 512s] tile, 5 fp16 matmuls (measured at the PE's
    throughput floor; fp8-DoubleRow loses its 2x to a serialized ~125ns
    LDWEIGHTS since 256-col dual-fp8 weights occupy both PE weight
    buffers, and matmul outputs are capped at 512 elements so the load
    cannot amortize over a wider moving stream):
      psum_ri[0:512]  = patR.psiR + patI.psiI        (Re)
      psum_ri[512:]   = patI.psiR + patR.(-psiI)     (Im)
      psum_l[si-half] = W.psiR                       (linear)
  - elementwise, per s-pair (s12q is a per-v-tile [128, 4, 1024] f16
    scratch holding [Re^2 | Im^2] per s-tile):
      s12q[s]  = Square(psum_ri)          (ACT -> f16; every Nth tile does
                                           DVE copy + f16 self-multiply
                                           instead, to balance engines)
      stage    = (psum_l + b_v) + s12q[Re-view]   (DVE stt, 1024 wide)
      stage   += s12q[Im-view]                    (DVE tt, f16 4x mode)
  - output written f16 ([6400, 2048] per core) halving write traffic;
    host upcasts to f32, transposes, concatenates, slices off padding.
"""

import sys

for _p in ("/opt/trn_rl_repo", "/opt/pypackages"):
    if _p not in sys.path:
        sys.path.append(_p)

import numpy as np

import concourse.bass as bass
import concourse.mybir as mybir
from concourse import bacc
from concourse.tile import TileContext
from concourse.bass_utils import run_bass_kernel_spmd


def _install_ntff_hook_shim():
    """Provide antenv.axon_hooks if the image lacks it, so trace=True can
    capture NTFF profiles through the axon PJRT .so."""
    try:
        from antenv import axon_hooks  # noqa: F401
        return
    except ImportError:
        pass
    import contextlib
    import ctypes
    import types

    import antenv

    so_path = "/opt/axon/libaxon_pjrt.so"
    mod = types.ModuleType("antenv.axon_hooks")
    _state = {"hook": None}

    def set_axon_ntff_profile_hook(hook):
        _state["hook"] = hook

    def get_axon_ntff_profile_hook():
        return _state["hook"]

    mod.set_axon_ntff_profile_hook = set_axon_ntff_profile_hook
    mod.get_axon_ntff_profile_hook = get_axon_ntff_profile_hook
    sys.modules["antenv.axon_hooks"] = mod
    antenv.axon_hooks = mod

    try:
        lib = ctypes.CDLL(so_path)
    except OSError:
        return
    if not hasattr(lib, "axon_start_nrt_profile"):
        return
    lib.axon_start_nrt_profile.argtypes = [
        ctypes.POINTER(ctypes.c_int64), ctypes.c_size_t]
    lib.axon_start_nrt_profile.restype = ctypes.c_int64
    lib.axon_stop_nrt_profile.argtypes = [ctypes.c_char_p]
    lib.axon_stop_nrt_profile.restype = ctypes.c_int64

    @contextlib.contextmanager
    def _hook(output_dir, device_ids):
        import jax
        jax.devices()
        if device_ids:
            ids = (ctypes.c_int64 * len(device_ids))(*device_ids)
            rc = lib.axon_start_nrt_profile(ids, len(device_ids))
        else:
            rc = lib.axon_start_nrt_profile(None, 0)
        if rc != 0:
            raise RuntimeError(f"axon_start_nrt_profile rc={rc}")
        try:
            yield
        finally:
            n = lib.axon_stop_nrt_profile(str(output_dir).encode())
            print(f"ntff profile: {n} file(s) written to {output_dir}",
                  file=sys.stderr)

    set_axon_ntff_profile_hook(_hook)


_install_ntff_hook_shim()

SEQ = 2048
EMBED = 128
VOCAB = 50257
N_CORES = 8
V_PAD = 51200            # 8 * 6400
V_CORE = V_PAD // N_CORES  # 6400
V_TILES = V_CORE // 128    # 50
S_TILE = 512
S_TILES = SEQ // S_TILE    # 4

F32 = mybir.dt.float32
F16 = mybir.dt.float16

# every DVE_SQ_MOD'th (v, s) tile squares its psum on DVE (copy + f16
# self-multiply) instead of ACT; 0 disables (DVE psum-copies release the
# psum banks slower than ACT squares, stalling the matmul pipeline)
DVE_SQ_MOD = 0
DVE_SQ_REM = 3

_compiled = {}


def _build_program():
    nc = bacc.Bacc()

    pat_r = nc.dram_tensor("pat_r", [EMBED, V_CORE], F16, kind="ExternalInput")
    pat_i = nc.dram_tensor("pat_i", [EMBED, V_CORE], F16, kind="ExternalInput")
    w_t = nc.dram_tensor("w_t", [EMBED, V_CORE], F16, kind="ExternalInput")
    psi_r = nc.dram_tensor("psi_r", [EMBED, SEQ], F16, kind="ExternalInput")
    psi_i = nc.dram_tensor("psi_i", [EMBED, SEQ], F16, kind="ExternalInput")
    psi_in = nc.dram_tensor("psi_in", [EMBED, SEQ], F16, kind="ExternalInput")
    b_rs = nc.dram_tensor("b_rs", [128, V_TILES], F32, kind="ExternalInput")
    out_t = nc.dram_tensor("out_t", [V_CORE, SEQ], F16, kind="ExternalOutput")

    add = mybir.AluOpType.add
    mult = mybir.AluOpType.mult

    with TileContext(nc) as tc:
        with tc.tile_pool(name="weights", bufs=1) as wpool, \
             tc.tile_pool(name="stage", bufs=3) as stpool, \
             tc.tile_pool(name="eltw", bufs=2) as epool, \
             tc.tile_pool(name="psum", bufs=2, space="PSUM") as pspool, \
             tc.tile_pool(name="psum_l", bufs=2, space="PSUM") as plpool:
            patr_sb = wpool.tile([EMBED, V_CORE], F16)
            pati_sb = wpool.tile([EMBED, V_CORE], F16)
            wt_sb = wpool.tile([EMBED, V_CORE], F16)
            psir_sb = wpool.tile([EMBED, SEQ], F16)
            psii_sb = wpool.tile([EMBED, SEQ], F16)
            psin_sb = wpool.tile([EMBED, SEQ], F16)
            b_sb = wpool.tile([128, V_TILES], F32)

            # load order: the first s-tile of psi + one v-tile of weights
            # unblock the first tile's matmuls within ~1us, then the rest
            sf = slice(0, S_TILE)
            nc.sync.dma_start(out=psir_sb[:, sf], in_=psi_r[:, sf])
            first = slice(0, 128)
            nc.sync.dma_start(out=patr_sb[:, first], in_=pat_r[:, first])
            nc.sync.dma_start(out=psii_sb[:, sf], in_=psi_i[:, sf])
            nc.sync.dma_start(out=pati_sb[:, first], in_=pat_i[:, first])
            nc.sync.dma_start(out=psin_sb[:, sf], in_=psi_in[:, sf])
            nc.sync.dma_start(out=wt_sb[:, first], in_=w_t[:, first])
            nc.sync.dma_start(out=b_sb[:], in_=b_rs[:])
            nc.sync.dma_start(out=psir_sb[:, S_TILE:], in_=psi_r[:, S_TILE:])
            nc.sync.dma_start(out=psii_sb[:, S_TILE:], in_=psi_i[:, S_TILE:])
            nc.sync.dma_start(out=psin_sb[:, S_TILE:], in_=psi_in[:, S_TILE:])
            first = slice(128, 2 * 128)
            nc.sync.dma_start(out=patr_sb[:, first], in_=pat_r[:, first])
            nc.sync.dma_start(out=pati_sb[:, first], in_=pat_i[:, first])
            nc.sync.dma_start(out=wt_sb[:, first], in_=w_t[:, first])
            CHUNK = 8 * 128
            for c0 in range(2 * 128, V_CORE, CHUNK):
                cs = slice(c0, min(c0 + CHUNK, V_CORE))
                nc.sync.dma_start(out=patr_sb[:, cs], in_=pat_r[:, cs])
                nc.sync.dma_start(out=pati_sb[:, cs], in_=pat_i[:, cs])
                nc.sync.dma_start(out=wt_sb[:, cs], in_=w_t[:, cs])

            for v in range(V_TILES):
                vs = slice(v * 128, (v + 1) * 128)
                stage = stpool.tile([128, S_TILES, S_TILE], F16, tag="stage")
                s12q = epool.tile([128, S_TILES, 2 * S_TILE], F16, tag="sq")
                for sp in range(S_TILES // 2):
                    psum_l = plpool.tile([128, 2, S_TILE], F32, tag="lin")
                    for si in range(2):
                        s = 2 * sp + si
                        ss = slice(s * S_TILE, (s + 1) * S_TILE)
                        psum_ri = pspool.tile([128, 2 * S_TILE], F32,
                                              tag="ri", name=f"ri_{v}_{s}")
                        nc.tensor.matmul(psum_ri[:, 0:S_TILE],
                                         patr_sb[:, vs], psir_sb[:, ss],
                                         start=True, stop=False)
                        nc.tensor.matmul(psum_ri[:, 0:S_TILE],
                                         pati_sb[:, vs], psii_sb[:, ss],
                                         start=False, stop=True)
                        nc.tensor.matmul(psum_ri[:, S_TILE:],
                                         pati_sb[:, vs], psir_sb[:, ss],
                                         start=True, stop=False)
                        nc.tensor.matmul(psum_ri[:, S_TILE:],
                                         patr_sb[:, vs], psin_sb[:, ss],
                                         start=False, stop=True)
                        nc.tensor.matmul(psum_l[:, si, :], wt_sb[:, vs],
                                         psir_sb[:, ss],
                                         start=True, stop=True)
                        if DVE_SQ_MOD and \
                                (v * S_TILES + s) % DVE_SQ_MOD == DVE_SQ_REM:
                            c_ri = epool.tile([128, 2 * S_TILE], F16,
                                              tag="cri")
                            nc.vector.tensor_copy(out=c_ri[:],
                                                  in_=psum_ri[:])
                            nc.vector.tensor_tensor(
                                out=s12q[:, s, :], in0=c_ri[:], in1=c_ri[:],
                                op=mult)
                        else:
                            nc.scalar.square(s12q[:, s, :], psum_ri[:])
                    pair = slice(2 * sp, 2 * sp + 2)
                    nc.vector.scalar_tensor_tensor(
                        out=stage[:, pair, :], in0=psum_l[:],
                        scalar=b_sb[:, v:v + 1],
                        in1=s12q[:, pair, 0:S_TILE],
                        op0=add, op1=add)
                    nc.vector.tensor_tensor(
                        out=stage[:, pair, :], in0=stage[:, pair, :],
                        in1=s12q[:, pair, S_TILE:], op=add)
                    nc.sync.dma_start(
                        out=out_t[vs, sp * 2 * S_TILE:(sp + 1) * 2 * S_TILE],
                        in_=stage[:, pair, :])

    nc.finalize()
    return nc


def _get_program():
    if "nc" not in _compiled:
        _compiled["nc"] = _build_program()
    return _compiled["nc"]


def kernel(psi_real, psi_imag, patterns_real, patterns_imag, W, b):
    psi_real = np.ascontiguousarray(psi_real, dtype=np.float32)
    psi_imag = np.ascontiguousarray(psi_imag, dtype=np.float32)

    psiT_r = np.ascontiguousarray(psi_real.T.astype(np.float16))
    psiT_i = np.ascontiguousarray(psi_imag.T.astype(np.float16))
    psiT_in = np.ascontiguousarray((-psi_imag.T).astype(np.float16))

    def pad_t(m):
        full = np.zeros((EMBED, V_PAD), dtype=np.float16)
        full[:, :VOCAB] = np.asarray(m, dtype=np.float32).T.astype(np.float16)
        return full

    patT_r = pad_t(patterns_real)
    patT_i = pad_t(patterns_imag)
    wT = pad_t(W)
    b_pad = np.zeros((V_PAD,), dtype=np.float32)
    b_pad[:VOCAB] = np.asarray(b, dtype=np.float32)

    in_maps = []
    for c in range(N_CORES):
        vs = slice(c * V_CORE, (c + 1) * V_CORE)
        b_shard = b_pad[vs]
        in_maps.append({
            "pat_r": np.ascontiguousarray(patT_r[:, vs]),
            "pat_i": np.ascontiguousarray(patT_i[:, vs]),
            "w_t": np.ascontiguousarray(wT[:, vs]),
            "psi_r": psiT_r,
            "psi_i": psiT_i,
            "psi_in": psiT_in,
            "b_rs": np.ascontiguousarray(b_shard.reshape(V_TILES, 128).T),
        })

    nc = _get_program()
    res = run_bass_kernel_spmd(nc, in_maps, core_ids=list(range(N_CORES)))
    kernel.last_results = res

    out = np.empty((SEQ, V_PAD), dtype=np.float32)
    for c in range(N_CORES):
        out[:, c * V_CORE:(c + 1) * V_CORE] = \
            res.results[c]["out_t"].T.astype(np.float32)
    return out[:, :VOCAB]
